# revision 4
# baseline (speedup 1.0000x reference)
"""Contrastive-loss kernel for Trainium2 (8 NeuronCores, data-parallel over batch).

Reference computation (B=64, S=64, F=4096, C=22):
    d[b,s]   = sum_f (xtes - x0es)^2
    cls      = argmax(yts, axis=-1); cls0 = cls[:, -1:]
    valid    = (cls != 21) & (cls0 != 21); same = cls == cls0
    loss     = sum(where(valid, where(same, d, relu(m - d)), 0)) / (B*S)

The 134 MB of xtes/x0es dominates (memory-bound); each core streams its
8-batch shard ([512, 4096] per tensor) and emits the 512 row distances.
The tiny yts argmax/masking and the final scalar reduction run on host.
"""

import sys

if "/opt/trn_rl_repo" not in sys.path:
    sys.path.insert(0, "/opt/trn_rl_repo")

import numpy as np

import concourse.bacc as bacc
import concourse.tile as tile
from concourse import mybir
from concourse.bass_utils import run_bass_kernel_spmd

IGNORE_INDEX = 21
B, S, F, C = 64, 64, 4096, 22
N_CORES = 8
BPC = B // N_CORES          # batches per core
ROWS = BPC * S              # 512 rows per core
P = 128                     # SBUF partitions
NT = ROWS // P              # 4 tiles of [128, F] per core

_nc = None                  # compiled-once Bass program
LAST_EXEC_TIME_NS = None    # filled when TRACE is on
TRACE = False


def _build():
    nc = bacc.Bacc(
        trn_type="TRN2",
        target_bir_lowering=False,
        debug=False,
        num_devices=N_CORES,
    )
    f32 = mybir.dt.float32
    x = nc.dram_tensor("x", [ROWS, F], f32, kind="ExternalInput").ap()
    x0 = nc.dram_tensor("x0", [ROWS, F], f32, kind="ExternalInput").ap()
    dout = nc.dram_tensor("dout", [P, NT], f32, kind="ExternalOutput").ap()

    X = x.rearrange("(t p) f -> t p f", p=P)
    X0 = x0.rearrange("(t p) f -> t p f", p=P)

    with tile.TileContext(nc) as tc:
        with (
            tc.tile_pool(name="io", bufs=3) as io_pool,
            tc.tile_pool(name="sq", bufs=2) as sq_pool,
            tc.tile_pool(name="acc", bufs=1) as acc_pool,
        ):
            dcol = acc_pool.tile([P, NT], f32)
            for t in range(NT):
                xt = io_pool.tile([P, F], f32)
                nc.sync.dma_start(xt[:], X[t])
                x0t = io_pool.tile([P, F], f32)
                nc.sync.dma_start(x0t[:], X0[t])
                # diff on DVE (in-place into xt), square+row-sum on ACT
                nc.vector.tensor_sub(xt[:], xt[:], x0t[:])
                sq = sq_pool.tile([P, F], f32)
                nc.scalar.activation(
                    sq[:],
                    xt[:],
                    mybir.ActivationFunctionType.Square,
                    accum_out=dcol[:, t : t + 1],
                )
            nc.sync.dma_start(dout[:], dcol[:])
    nc.compile()
    return nc


def kernel(xtes, x0es, yts, m):
    global _nc, LAST_EXEC_TIME_NS
    if _nc is None:
        _nc = _build()

    xtes = np.ascontiguousarray(np.asarray(xtes, dtype=np.float32)).reshape(B, S, F)
    x0es = np.ascontiguousarray(np.asarray(x0es, dtype=np.float32)).reshape(B, S, F)
    yts = np.asarray(yts)
    mf = float(np.asarray(m))

    in_maps = [
        {
            "x": np.ascontiguousarray(
                xtes[i * BPC : (i + 1) * BPC].reshape(ROWS, F)
            ),
            "x0": np.ascontiguousarray(
                x0es[i * BPC : (i + 1) * BPC].reshape(ROWS, F)
            ),
        }
        for i in range(N_CORES)
    ]

    res = run_bass_kernel_spmd(
        _nc, in_maps, core_ids=list(range(N_CORES)), trace=TRACE
    )
    LAST_EXEC_TIME_NS = res.exec_time_ns

    # dout[p, t] = d[row t*128+p] of that core's shard
    d = np.concatenate(
        [res.results[i]["dout"].T.reshape(ROWS) for i in range(N_CORES)]
    ).reshape(B, S)

    cls = np.argmax(np.asarray(yts, dtype=np.float32), axis=-1)
    cls0 = cls[:, -1:]
    valid = (cls != IGNORE_INDEX) & (cls0 != IGNORE_INDEX)
    same = cls == cls0
    per = np.where(same, d, np.maximum(np.float32(mf) - d, np.float32(0.0)))
    loss = np.where(valid, per, np.float32(0.0)).sum(dtype=np.float64) / (B * S)
    return np.float32(loss)
